# revision 23
# baseline (speedup 1.0000x reference)
"""Block-diagonal attention kernel for Trainium2 (8 NeuronCores).

Problem: q,k,v [4, 16, 4128, 64] f32. For each (b,h): attention is computed
independently within consecutive 64-row blocks (64 full blocks) plus one
final 32-row block (4128 = 64*64 + 32).

Sharding: B*H = 64 (b,h) pairs -> 8 pairs per core (pure data parallel).

Design notes (what made this fast):

1. DMA layout. The kernel is DMA-descriptor-bound if SBUF layouts force
   row-per-partition (256B) descriptors: <512B descriptors pay a 2x DMA
   latency penalty. kernel() owns the host-side sharding, so all tensors
   are relaid out on the host so every device DMA is flat with >=4KB
   contiguous per partition:
     - qkc: Q^T and K^T pair-stacked [128=(chunk parity, d), qk, pair, i].
       The S^T matmul needs d on partitions for both operands; hosting the
       transpose removes all PE transposes + their PSUM->SBUF copies.
     - vc: V natural-chunked [128, 32, 65] with the softmax-denominator
       ones column baked in (PV matmul computes row sums for free).
     - oc: output written partition-major [128, 32, 64]; host inverts.

2. fp16 everywhere: inputs are N(0,1) (|x| < ~7, exp(s/8) in [2e-3, 1e3] --
   normal fp16 range), matmuls run at 1 cycle/row vs fp32's 4, bytes halve.
   PSUM accumulation stays f32. Measured rel err ~5e-4 vs the 2e-2 gate.

3. Aligned-quadrant blocking. Computing S^T per 128-row chunk as one
   [128,128] matmul leaves the two diagonal 64x64 blocks at different
   columns per partition half, forcing 2 activation instrs + 2 memsets per
   superchunk (ACT-instruction-bound: ~150ns fixed cost each). Instead each
   64-block gets its own matmul; tile_position=(row, col) places block B's
   S^T at partitions 64:128 on the SAME columns as block A's at 0:64. Then:
     - ONE exp per superchunk ([128, 256]: all 8 blocks' scores),
     - no memsets (no off-diagonal garbage is ever materialized),
     - per-block PV with tile_position (0,0)/(64,64) into one PSUM tile,
     - ONE reciprocal + ONE broadcast tensor_tensor multiply per superchunk
       to normalize (r [128,4] broadcast along d).

Per-superchunk engine cost: PE 16 matmuls (same cycles as 8 bigger ones),
ACT 1 exp, DVE 1 recip + 1 tensor_mul, no Pool compute. DMA instrs ride
SP (qk) and Pool SWDGE (v, out), keeping ACT pure compute.
"""
import sys

sys.path.insert(0, "/opt/trn_rl_repo")

import numpy as np
from contextlib import ExitStack

import concourse.tile as tile
from concourse import bacc, mybir
from concourse.bass_utils import run_bass_kernel_spmd

F32 = mybir.dt.float32
F16 = mybir.dt.float16
AF = mybir.ActivationFunctionType

B, H, N, D = 4, 16, 4128, 64
BHPC = 8                 # (b,h) pairs per core
NMAIN = 4096             # rows covered by full 64-blocks
NREM = 32                # remainder block rows
N_SC = 8                 # superchunks (512 rows) per (b,h)
SCALE = 1.0 / 8.0        # 1/sqrt(D)
NPDT = np.float16

PSUM_BUFS = {"ss": 2, "o": 2}
PT_BUFS = 3
BIG_BUFS = 6
SB_BUFS = 3


def _superchunk(nc, sb, ps, qksb, vsb, outb, s):
    # S^T per 64-block: chunk parity selects the partition half (=PE row
    # group) holding that chunk's Q^T/K^T; block A -> psum partitions 0:64,
    # block B -> 64:128 at the SAME columns (tile_position col offset).
    # Even/odd chunks write different PSUM banks (cols 0:128 vs 512:640):
    # concurrent row-group-tiled matmuls must not share a bank.
    ss = ps.tile([128, 1024], F32, tag="ss", bufs=PSUM_BUFS["ss"])
    for c in range(4):
        par = c % 2
        g = c // 2
        pair = 2 * s + g
        p0 = 64 * par
        col = 512 * par + 64 * g
        kT = qksb[p0:p0 + 64, 1, pair, :]
        qT = qksb[p0:p0 + 64, 0, pair, :]
        nc.tensor.matmul(ss[0:64, col:col + 64], kT[:, 0:64], qT[:, 0:64],
                         tile_position=(p0, 0))
        nc.tensor.matmul(ss[64:128, col:col + 64], kT[:, 64:128], qT[:, 64:128],
                         tile_position=(p0, 64))

    # ONE exp for all 8 blocks of the superchunk
    pt = sb.tile([128, 2, 2, 64], F16, tag="pt", bufs=PT_BUFS)
    ssq = ss.rearrange("p (par x g d) -> p par x g d", par=2, x=4, g=2)[:, :, 0]
    nc.scalar.activation(pt[:], ssq, AF.Exp, scale=SCALE)

    # PV per block: o[i-half, c, 0:65] = P_blk^T.T @ [V_blk | 1]
    # (row stride 128 f32 so no matmul write crosses a PSUM bank)
    o = ps.tile([128, 4, 128], F32, tag="o", bufs=PSUM_BUFS["o"])
    for c in range(4):
        par = c % 2
        g = c // 2
        ci = 4 * s + c
        nc.tensor.matmul(o[0:64, c, 0:65], pt[0:64, par, g, :],
                         vsb[0:64, ci, :], tile_position=(0, 0))
        nc.tensor.matmul(o[64:128, c, 0:65], pt[64:128, par, g, :],
                         vsb[64:128, ci, :], tile_position=(64, 64))

    # normalize all 4 chunks at once: r broadcast along d
    r = sb.tile([128, 4], F32, tag="r")
    nc.vector.reciprocal(r[:], o[:, :, 64])
    rb = r[:, :, None].broadcast_to([128, 4, 64])
    nc.vector.tensor_mul(outb[:, 4 * s:4 * s + 4, :], o[:, :, 0:64], rb)


def _remainder(nc, sb, ps, rqksb, rvsb, roc):
    """All 8 bh remainder blocks ([32, 64] each) in one pass on partitions
    0:32, blocks stacked along the free dim. All matmuls share row group ->
    serialized -> single-bank PSUM writes are safe."""
    rss = ps.tile([32, 8, 32], F32, tag="o", bufs=PSUM_BUFS["o"])
    for j in range(8):
        nc.tensor.matmul(rss[:, j, :], rqksb[:, 1, j, :], rqksb[:, 0, j, :])

    rpt = sb.tile([32, 8, 32], F16, tag="rpt")
    nc.scalar.activation(rpt[:], rss[:], AF.Exp, scale=SCALE)

    # PV per block: [32, 65] at cols 128j of a 2-bank tile (no crossing)
    ro = ps.tile([32, 8, 128], F32, tag="ss", bufs=PSUM_BUFS["ss"])
    for j in range(8):
        nc.tensor.matmul(ro[:, j, 0:65], rpt[:, j, :], rvsb[:, j, :])

    rr = sb.tile([32, 8], F32, tag="rr")
    nc.vector.reciprocal(rr[:], ro[:, :, 64])
    routs = sb.tile([32, 8, 64], F16, tag="routs")
    rrb = rr[:, :, None].broadcast_to([32, 8, 64])
    nc.vector.tensor_mul(routs[:], ro[:, :, 0:64], rrb)

    nc.gpsimd.dma_start(out=roc[:], in_=routs[:])


def build_nc(repeat=1):
    nc = bacc.Bacc("TRN2", target_bir_lowering=False, debug=False, num_devices=8)
    qkc = nc.dram_tensor("qkc", [BHPC, 128, 2, 16, 128], F16,
                         kind="ExternalInput").ap()
    vc = nc.dram_tensor("vc", [BHPC, 128, 32, 65], F16,
                        kind="ExternalInput").ap()
    rqk = nc.dram_tensor("rqk", [64, 2, BHPC, 32], F16,
                         kind="ExternalInput").ap()
    rv = nc.dram_tensor("rv", [32, BHPC, 65], F16, kind="ExternalInput").ap()
    oc = nc.dram_tensor("oc", [BHPC, 128, 32, 64], F16,
                        kind="ExternalOutput").ap()
    roc = nc.dram_tensor("roc", [32, BHPC, 64], F16, kind="ExternalOutput").ap()

    with tile.TileContext(nc) as tc, ExitStack() as ctx:
        singles = ctx.enter_context(tc.tile_pool(name="singles", bufs=1))
        big = ctx.enter_context(tc.tile_pool(name="big", bufs=BIG_BUFS))
        sb = ctx.enter_context(tc.tile_pool(name="sb", bufs=SB_BUFS))
        ps = ctx.enter_context(tc.tile_pool(name="ps", bufs=2, space="PSUM"))

        # remainder inputs are tiny; load once at startup
        rqksb = singles.tile([64, 2, BHPC, 32], F16)
        rvsb = singles.tile([32, BHPC, 65], F16)
        nc.gpsimd.dma_start(out=rqksb[:], in_=rqk[:])
        nc.gpsimd.dma_start(out=rvsb[:], in_=rv[:])

        for _ in range(repeat):
            for bh in range(BHPC):
                # flat whole-head loads: one >=4KB descriptor per partition
                qksb = big.tile([128, 2, 16, 128], F16, tag="qksb")
                vsb = big.tile([128, 32, 65], F16, tag="vsb")
                nc.sync.dma_start(out=qksb[:], in_=qkc[bh])
                nc.gpsimd.dma_start(out=vsb[:], in_=vc[bh])
                outb = big.tile([128, 32, 64], F16, tag="outb")
                for s in range(N_SC):
                    _superchunk(nc, sb, ps, qksb, vsb, outb, s)
                nc.gpsimd.dma_start(out=oc[bh], in_=outb[:])
            _remainder(nc, sb, ps, rqksb, rvsb, roc)

    nc.compile()
    return nc


def build_in_maps(q, k, v):
    """Host-side relayout: full f32 inputs -> per-core fp16 flat tensors."""
    q64 = np.asarray(q, np.float32).reshape(B * H, N, D)
    k64 = np.asarray(k, np.float32).reshape(B * H, N, D)
    v64 = np.asarray(v, np.float32).reshape(B * H, N, D)

    # qkc[bh, par*64+d, qk, pair, i] = T[qk][bh, 256*pair + 128*par + i, d]
    qm = q64[:, :NMAIN, :].astype(NPDT).reshape(64, 16, 2, 128, 64)
    km = k64[:, :NMAIN, :].astype(NPDT).reshape(64, 16, 2, 128, 64)
    qT = qm.transpose(0, 2, 4, 1, 3)          # bh, par, d, pair, i
    kT = km.transpose(0, 2, 4, 1, 3)
    qkc = np.ascontiguousarray(
        np.stack([qT, kT], axis=3)).reshape(64, 128, 2, 16, 128)

    # vc[bh, p, ci, 0:64] = v[bh, 128*ci + p, :]; col 64 = ones
    vm = v64[:, :NMAIN, :].astype(NPDT).reshape(64, 32, 128, 64)
    vc = np.empty((64, 128, 32, 65), dtype=NPDT)
    vc[:, :, :, :64] = vm.transpose(0, 2, 1, 3)
    vc[:, :, :, 64] = 1.0

    # remainder: rqk[core][d, qk, j(bh-in-core), i], rv[core][i, j, 0:65]
    qr = q64[:, NMAIN:, :].astype(NPDT)       # [64, 32, 64]
    kr = k64[:, NMAIN:, :].astype(NPDT)
    vr = v64[:, NMAIN:, :].astype(NPDT)

    in_maps = []
    for i in range(8):
        sl = slice(BHPC * i, BHPC * (i + 1))
        rqk = np.ascontiguousarray(
            np.stack([qr[sl].transpose(2, 0, 1), kr[sl].transpose(2, 0, 1)],
                     axis=1))                 # [64, 2, 8, 32]
        rvc = np.empty((32, BHPC, 65), dtype=NPDT)
        rvc[:, :, :64] = vr[sl].transpose(1, 0, 2)
        rvc[:, :, 64] = 1.0
        in_maps.append({
            "qkc": np.ascontiguousarray(qkc[sl]),
            "vc": np.ascontiguousarray(vc[sl]),
            "rqk": rqk,
            "rv": rvc,
        })
    return in_maps


def assemble_output(results):
    """Per-core {oc, roc} -> full [B, H, N, D] f32."""
    out = np.empty((B * H, N, D), dtype=np.float32)
    for i in range(8):
        sl = slice(BHPC * i, BHPC * (i + 1))
        oc = np.asarray(results[i]["oc"], dtype=np.float32)   # [8, 128, 32, 64]
        roc = np.asarray(results[i]["roc"], dtype=np.float32)  # [32, 8, 64]
        out[sl, :NMAIN, :] = oc.transpose(0, 2, 1, 3).reshape(BHPC, NMAIN, D)
        out[sl, NMAIN:, :] = roc.transpose(1, 0, 2)
    return out.reshape(B, H, N, D)


_CACHE = {}


def kernel(q, k, v):
    assert q.shape == (B, H, N, D), q.shape
    if "nc" not in _CACHE:
        _CACHE["nc"] = build_nc()
    nc = _CACHE["nc"]

    in_maps = build_in_maps(q, k, v)

    # One retry: rapid repeated executions occasionally wedge a core with a
    # transient NRT_EXEC_UNIT_UNRECOVERABLE; a fresh attempt recovers.
    try:
        res = run_bass_kernel_spmd(nc, in_maps, core_ids=list(range(8)))
    except Exception:
        import time
        time.sleep(2.0)
        res = run_bass_kernel_spmd(nc, in_maps, core_ids=list(range(8)))
    return assemble_output(res.results)
